# revision 1
# baseline (speedup 1.0000x reference)
"""Trainium2 Bass kernel for a transformer decoder layer (self-attn + cross-attn + FFN,
3 LayerNorms). Data-parallel over batch: 8 batch elements -> 8 NeuronCores, no collectives.

Per-core dataflow (one batch element, S=512, D=1024, H=16, HD=64, DFF=4096):
  - Activations live feature-major in SBUF: X^T [D, T] as tiles [128, D/128, T].
  - Projections: out X'^T[mc] = sum_kc W[kc,mc].T @ X^T[kc]  (weights stationary).
  - Scores computed transposed: s^T[s,t] = k_h^T(.,s).T @ q_h^T  (K=HD=64).
  - Softmax without max-subtraction (scores are O(1); masked entries get -1e5 -> exp==0).
    Denominator via a ones-column prepended to V in the AV matmul (psum row 0).
  - AV: bU^T[hd,t] = V_aug[s,:].T @ exp(s^T), normalize with partition-broadcast 1/denom.
  - LayerNorm feature-major: stats over partitions via ones-vector matmuls, apply with
    partition-broadcast mean/rstd.
All host-side reshapes/transposes (inputs, weights, output) are in kernel() below.
"""
import numpy as np

B, S, D, H, HD, DFF = 8, 512, 1024, 16, 64, 4096
KC = D // 128    # 8 feature chunks
SC = S // 128    # 4 sequence chunks
FC = DFF // 128  # 32 ffn chunks
QSCALE = float(1.0 / (np.sqrt(np.float32(1024.0)) + 1e-8))
NEGBIG = -6.0e4  # fits fp16
LN_EPS = 1e-3

_CACHE = {}


def _build_nc(phases=6, look=2, pmmb=5, pavb=3, loop_n=1):
    import concourse.mybir as mybir
    import concourse.tile as tile
    from concourse import bacc

    F32 = mybir.dt.float32
    F32R = mybir.dt.float32r
    F16 = mybir.dt.float16
    AF = mybir.ActivationFunctionType
    OP = mybir.AluOpType

    nc = bacc.Bacc("TRN2", target_bir_lowering=False, debug=False, num_devices=8)

    # ---- DRAM I/O ----
    yzT = nc.dram_tensor("yzT", [128, KC, S], F16, kind="ExternalInput")
    xzT = nc.dram_tensor("xzT", [128, KC, S], F16, kind="ExternalInput")
    maskT_d = nc.dram_tensor("maskT", [128, SC, S], F16, kind="ExternalInput")
    wdec = {}
    for nm in ("wq_s", "wk_s", "wo_s", "wq_c", "wk_c", "wo_c"):
        wdec[nm] = nc.dram_tensor(nm, [KC, 128, KC, 128], F16, kind="ExternalInput")
    for nm in ("wv_s", "wv_c"):
        wdec[nm] = nc.dram_tensor(nm, [KC, 128, D], F16, kind="ExternalInput")
    w1_d = nc.dram_tensor("w1", [FC, 128, KC, 128], F16, kind="ExternalInput")
    w2_d = nc.dram_tensor("w2", [KC, 128, FC, 128], F16, kind="ExternalInput")
    consts_d = nc.dram_tensor("consts", [128, FC + KC * 7 + SC], F32,
                              kind="ExternalInput")
    outT_d = nc.dram_tensor("outT", [128, KC, S], F16, kind="ExternalOutput")

    with tile.TileContext(nc) as tc:
        with tc.tile_pool(name="persist", bufs=1) as persist:
            def _iter_body(_it=None):
                # ---- constants / persistent activations ----
                ones_t = persist.tile([128, 1], F16, tag="ones", name="ones_t")
                nc.vector.memset(ones_t[:], 1.0)
                eps_t = persist.tile([1, 1], F32, tag="eps", name="eps_t")
                nc.vector.memset(eps_t[:], LN_EPS)
                yzT_t = persist.tile([128, KC, S], F16, tag="yzT", name="yzT_t")
                nc.scalar.dma_start(out=yzT_t[:], in_=yzT[:, :, :])
                consts_t = persist.tile([128, FC + KC * 7 + SC], F32, tag="consts",
                                        name="consts_t")
                nc.scalar.dma_start(out=consts_t[:], in_=consts_d[:, :])
                b1_t = consts_t[:, 0:FC]
                b2_t = consts_t[:, FC:FC + KC]
                ln_t = {}
                for i, nm in enumerate(("g0", "be0", "g1", "be1", "g2", "be2")):
                    off = FC + KC + i * KC
                    ln_t[nm] = consts_t[:, off:off + KC]
                padb_t = consts_t[:, FC + KC * 7:FC + KC * 7 + SC]
                xzT_t = persist.tile([128, KC, S], F16, tag="xzT", name="xzT_t")
                o1T_t = persist.tile([128, KC, S], F16, tag="o1T", name="o1T_t")
                o2T_t = persist.tile([128, KC, S], F16, tag="o2T", name="o2T_t")
                resT_t = persist.tile([128, KC, S], F16, tag="resT", name="resT_t")

                # ---------------- helpers ----------------
                def emit_attention(idx, qsrcT, kvT, wq, wk, wv, wo, is_self, residT):
                    """Writes resT_t = attn_out + residT (feature-major chunks)."""
                    with tc.tile_pool(name=f"attn{idx}", bufs=1) as ap, \
                         tc.tile_pool(name=f"attn{idx}_a", bufs=8) as apool, \
                         tc.tile_pool(name=f"attn{idx}_w", bufs=2) as wpool, \
                         tc.tile_pool(name=f"attn{idx}_s", bufs=6) as spool, \
                         tc.tile_pool(name=f"attn{idx}_d", bufs=6) as dnmp, \
                         tc.tile_pool(name=f"attn{idx}_pm", bufs=pmmb, space="PSUM") as pmm, \
                         tc.tile_pool(name=f"attn{idx}_pav", bufs=pavb, space="PSUM") as pavp:
                        QT = ap.tile([128, KC, S], F16, tag="QT", name=f"QT{idx}")
                        KT = ap.tile([128, KC, S], F16, tag="KT", name=f"KT{idx}")
                        Vt = ap.tile([128, SC, H, HD + 1], F16, tag="Vt", name=f"Vt{idx}")
                        bT = ap.tile([128, KC, S], F16, tag="bT", name=f"bT{idx}")

                        # K and V first (kvT-dependent; overlaps with the LN
                        # producing qsrcT for cross-attention), then Q.
                        # V: normal layout [s, (h hd)] with ones column at 64
                        for j in range(SC):
                            nc.vector.memset(Vt[:, j, :, HD:HD + 1], 1.0)
                        wvt = wpool.tile([128, KC, D], F16, tag="wmv",
                                         name=f"wv{idx}")
                        nc.sync.dma_start(out=wvt[:], in_=wv[:, :, :].rearrange(
                            "k p m -> p k m"))
                        for nh in range(2):
                            for j in range(SC):
                                pv = pmm.tile([128, S], F32, tag="pmm",
                                              name=f"pv{idx}_{nh}_{j}")
                                for kc in range(KC):
                                    nc.tensor.matmul(
                                        pv[:], kvT[:, kc, 128 * j:128 * (j + 1)],
                                        wvt[:, kc, 512 * nh:512 * (nh + 1)],
                                        start=(kc == 0), stop=(kc == KC - 1))
                                # psum [128, 512] -> V[:, j, 8nh:8nh+8, 0:64]
                                nc.scalar.activation(
                                    out=Vt[:, j, 8 * nh:8 * (nh + 1), 0:HD],
                                    in_=pv[:], func=AF.Copy)
                        for which, wsrc, src, dst in (("k", wk, kvT, KT),
                                                      ("q", wq, qsrcT, QT)):
                            wt = wpool.tile([128, KC, KC, 128], F16, tag="wst",
                                            name=f"w{which}{idx}")
                            nc.sync.dma_start(out=wt[:], in_=wsrc[:, :, :, :].rearrange(
                                "mc p kc m -> p mc kc m"))
                            for mc in range(KC):
                                pq = pmm.tile([128, S], F32, tag="pmm",
                                              name=f"p{which}{idx}_{mc}")
                                for kc in range(KC):
                                    nc.tensor.matmul(pq[:], wt[:, mc, kc, :],
                                                     src[:, kc, :],
                                                     start=(kc == 0), stop=(kc == KC - 1))
                                if which == "q":
                                    nc.scalar.activation(out=dst[:, mc, :], in_=pq[:],
                                                         func=AF.Copy, scale=QSCALE)
                                else:
                                    nc.scalar.activation(out=dst[:, mc, :], in_=pq[:],
                                                         func=AF.Copy)

                        # per-head scores + AV, software-pipelined one head ahead
                        # so the in-order PE never waits on the exp/mask chain.
                        aTs = {}

                        def emit_scores(h):
                            mc, off = h // 2, 64 * (h % 2)
                            for j in range(SC):
                                pst = pmm.tile([128, S], F32, tag="pmm",
                                               name=f"ps{idx}_{h}_{j}")
                                nc.tensor.matmul(
                                    pst[:],
                                    KT[off:off + 64, mc, 128 * j:128 * (j + 1)],
                                    QT[off:off + 64, mc, :],
                                    start=True, stop=True)
                                aT = apool.tile([128, S], F16, tag="aT",
                                                name=f"aT{idx}_{h}_{j}")
                                if is_self:
                                    nc.scalar.activation(out=aT[:], in_=pst[:], func=AF.Exp)
                                    eng = nc.vector if (h % 8 < 5) else nc.gpsimd
                                    eng.tensor_mul(out=aT[:], in0=aT[:],
                                                   in1=maskT_t[:, j, :])
                                else:
                                    nc.scalar.activation(out=aT[:], in_=pst[:], func=AF.Exp,
                                                         bias=padb_t[:, j:j + 1])
                                aTs[(h, j)] = aT

                        def emit_av(h):
                            mc, off = h // 2, 64 * (h % 2)
                            pav = pavp.tile([HD + 1, S], F32, tag="pav",
                                            name=f"pav{idx}_{h}")
                            for j in range(SC):
                                nc.tensor.matmul(pav[:], Vt[:, j, h, :], aTs.pop((h, j))[:],
                                                 start=(j == 0), stop=(j == SC - 1))
                            dnm = dnmp.tile([1, S], F16, tag="dnm", name=f"dnm{idx}_{h}")
                            nc.scalar.activation(out=dnm[:], in_=pav[HD:HD + 1, :],
                                                 func=AF.Copy)
                            rb = spool.tile([64, S], F16, tag="rb", name=f"rb{idx}_{h}")
                            nc.gpsimd.partition_broadcast(rb[:], dnm[:])
                            with nc.allow_low_precision(reason="softmax denom recip fp16"):
                                nc.vector.reciprocal(out=rb[:], in_=rb[:])
                            nc.vector.tensor_mul(out=bT[off:off + 64, mc, :],
                                                 in0=pav[0:HD, :], in1=rb[:])

                        for h in range(look):
                            emit_scores(h)
                        for h in range(H):
                            if h + look < H:
                                emit_scores(h + look)
                            emit_av(h)

                        # output projection + residual add
                        wt = wpool.tile([128, KC, KC, 128], F16, tag="wst",
                                        name=f"wo{idx}")
                        nc.sync.dma_start(out=wt[:], in_=wo[:, :, :, :].rearrange(
                            "mc p kc m -> p mc kc m"))
                        for mc in range(KC):
                            po = pmm.tile([128, S], F32, tag="pmm", name=f"po{idx}_{mc}")
                            for kc in range(KC):
                                nc.tensor.matmul(po[:], wt[:, mc, kc, :], bT[:, kc, :],
                                                 start=(kc == 0), stop=(kc == KC - 1))
                            nc.vector.tensor_add(out=resT_t[:, mc, :], in0=po[:],
                                                 in1=residT[:, mc, :])

                def emit_ln(idx, g_ap, b_ap, outT):
                    """LayerNorm over features of resT_t -> outT ([128, KC, S] tile or
                    None to stream to DRAM output)."""
                    with tc.tile_pool(name=f"ln{idx}_ps", bufs=4, space="PSUM") as pstat, \
                         tc.tile_pool(name=f"ln{idx}_st", bufs=3) as stage, \
                         tc.tile_pool(name=f"ln{idx}_x", bufs=KC) as lnx, \
                         tc.tile_pool(name=f"ln{idx}_sm", bufs=1) as lnsm:
                        # two parallel accumulation chains per stat to cut latency
                        psms = [pstat.tile([1, S], F32, tag="psm", name=f"psm{idx}_{i}")
                                for i in range(2)]
                        psss = [pstat.tile([1, S], F32, tag="psm", name=f"pss{idx}_{i}")
                                for i in range(2)]
                        xsqs = []
                        for kc in range(KC):
                            i, last = kc % 2, kc >= KC - 2
                            nc.tensor.matmul(psms[i][:], ones_t[:], resT_t[:, kc, :],
                                             start=(kc < 2), stop=last)
                            xsq = lnx.tile([128, S], F16, tag="xsq",
                                           name=f"xsq{idx}_{kc}")
                            nc.scalar.activation(out=xsq[:], in_=resT_t[:, kc, :],
                                                 func=AF.Square)
                            xsqs.append(xsq)
                        for kc in range(KC):
                            i, last = kc % 2, kc >= KC - 2
                            # xsq tiles cycle through 2 stage bufs; re-derive handle
                            nc.tensor.matmul(psss[i][:], ones_t[:],
                                             xsqs[kc][:],
                                             start=(kc < 2), stop=last)
                        m1 = lnsm.tile([1, S], F32, tag="m1", name=f"m1_{idx}")
                        nc.scalar.activation(out=m1[:], in_=psms[1][:], func=AF.Copy)
                        mrow = lnsm.tile([1, S], F32, tag="mrow", name=f"mrow{idx}")
                        nc.vector.tensor_add(out=mrow[:], in0=psms[0][:], in1=m1[:])
                        e1 = lnsm.tile([1, S], F32, tag="e1", name=f"e1_{idx}")
                        nc.scalar.activation(out=e1[:], in_=psss[1][:], func=AF.Copy)
                        erow = lnsm.tile([1, S], F32, tag="erow", name=f"erow{idx}")
                        nc.vector.tensor_add(out=erow[:], in0=psss[0][:], in1=e1[:])
                        nc.scalar.activation(out=mrow[:], in_=mrow[:], func=AF.Copy,
                                             scale=1.0 / D)
                        nc.scalar.activation(out=erow[:], in_=erow[:], func=AF.Copy,
                                             scale=1.0 / D)
                        vrow = lnsm.tile([1, S], F32, tag="vrow", name=f"vrow{idx}")
                        nc.vector.tensor_mul(out=vrow[:], in0=mrow[:], in1=mrow[:])
                        nc.vector.tensor_sub(out=vrow[:], in0=erow[:], in1=vrow[:])
                        nc.scalar.activation(out=vrow[:], in_=vrow[:], func=AF.Sqrt,
                                             bias=eps_t[:])
                        nc.vector.reciprocal(out=vrow[:], in_=vrow[:])
                        mrow16 = lnsm.tile([1, S], F16, tag="mrow16", name=f"mrow16{idx}")
                        nc.scalar.activation(out=mrow16[:], in_=mrow[:], func=AF.Copy)
                        vrow16 = lnsm.tile([1, S], F16, tag="vrow16", name=f"vrow16{idx}")
                        nc.scalar.activation(out=vrow16[:], in_=vrow[:], func=AF.Copy)
                        meanB = stage.tile([128, S], F16, tag="lnB", name=f"meanB{idx}")
                        nc.gpsimd.partition_broadcast(meanB[:], mrow16[:])
                        rstdB = stage.tile([128, S], F16, tag="lnB", name=f"rstdB{idx}")
                        nc.gpsimd.partition_broadcast(rstdB[:], vrow16[:])
                        for kc in range(KC):
                            dte = stage.tile([128, S], F16, tag="lnd", name=f"lnd{idx}_{kc}")
                            eng = nc.vector if kc % 4 != 3 else nc.gpsimd
                            eng.tensor_sub(out=dte[:], in0=resT_t[:, kc, :],
                                           in1=meanB[:])
                            nc.vector.scalar_tensor_tensor(
                                out=dte[:], in0=dte[:], scalar=g_ap[:, kc:kc + 1],
                                in1=rstdB[:], op0=OP.mult, op1=OP.mult)
                            if outT is not None:
                                nc.scalar.activation(out=outT[:, kc, :], in_=dte[:],
                                                     func=AF.Identity,
                                                     bias=b_ap[:, kc:kc + 1])
                            else:
                                ote = stage.tile([128, S], F16, tag="lno",
                                                 name=f"lno{idx}_{kc}")
                                nc.scalar.activation(out=ote[:], in_=dte[:],
                                                     func=AF.Identity,
                                                     bias=b_ap[:, kc:kc + 1])
                                nc.scalar.dma_start(out=outT_d[:, kc, :], in_=ote[:])

                def emit_ffn():
                    """resT_t = relu(o2T @ w1 + b1) @ w2 + b2 + o2T"""
                    with tc.tile_pool(name="ffn", bufs=1) as fp, \
                         tc.tile_pool(name="ffn_w", bufs=3) as fw, \
                         tc.tile_pool(name="ffn_w2", bufs=2) as fw2, \
                         tc.tile_pool(name="ffn_ps", bufs=4, space="PSUM") as pf:
                        hT = fp.tile([128, FC, S], F16, tag="hT", name="hT")
                        for g in range(FC // 4):
                            wt = fw.tile([128, 4, KC, 128], F16, tag="w1t", name=f"w1_{g}")
                            nc.sync.dma_start(out=wt[:], in_=w1_d[4 * g:4 * (g + 1), :, :, :]
                                              .rearrange("mc p kc m -> p mc kc m"))
                            for mi in range(4):
                                mc = 4 * g + mi
                                ph = pf.tile([128, S], F32, tag="pf", name=f"ph{mc}")
                                for kc in range(KC):
                                    nc.tensor.matmul(ph[:], wt[:, mi, kc, :],
                                                     o2T_t[:, kc, :],
                                                     start=(kc == 0), stop=(kc == KC - 1))
                                nc.scalar.activation(out=hT[:, mc, :], in_=ph[:],
                                                     func=AF.Relu, bias=b1_t[:, mc:mc + 1])
                        for mc in range(KC):
                            wt = fw2.tile([128, FC, 128], F16, tag="w2t", name=f"w2_{mc}")
                            nc.sync.dma_start(out=wt[:], in_=w2_d[mc, :, :, :])
                            po = pf.tile([128, S], F32, tag="pf", name=f"po2_{mc}")
                            for kc in range(FC):
                                nc.tensor.matmul(po[:], wt[:, kc, :], hT[:, kc, :],
                                                 start=(kc == 0), stop=(kc == FC - 1))
                            # resT = (po + b2) + o2T
                            nc.vector.scalar_tensor_tensor(
                                out=resT_t[:, mc, :], in0=po[:], scalar=b2_t[:, mc:mc + 1],
                                in1=o2T_t[:, mc, :], op0=OP.add, op1=OP.add)

                # ---------------- main flow ----------------
                with tc.tile_pool(name="maskp", bufs=1) as maskp:
                    maskT_t = maskp.tile([128, SC, S], F16, tag="maskT", name="maskT_t")
                    nc.scalar.dma_start(out=maskT_t[:], in_=maskT_d[:, :, :])

                    if phases >= 1:
                        emit_attention(0, yzT_t, yzT_t, wdec["wq_s"], wdec["wk_s"],
                                       wdec["wv_s"], wdec["wo_s"], True, yzT_t)
                    if phases >= 2:
                        emit_ln(0, ln_t["g0"], ln_t["be0"], o1T_t)
                    nc.scalar.dma_start(out=xzT_t[:], in_=xzT[:, :, :])
                    if phases >= 3:
                        emit_attention(1, o1T_t, xzT_t, wdec["wq_c"], wdec["wk_c"],
                                       wdec["wv_c"], wdec["wo_c"], False, o1T_t)
                    if phases >= 4:
                        emit_ln(1, ln_t["g1"], ln_t["be1"], o2T_t)
                if phases >= 5:
                    emit_ffn()
                if phases >= 6:
                    emit_ln(2, ln_t["g2"], ln_t["be2"], None)


            if loop_n == 1:
                _iter_body()
            else:
                with tc.For_i(0, loop_n, 1) as _it:
                    _iter_body(_it)

    nc.compile()
    return nc


def _get_nc():
    if "nc" not in _CACHE:
        _CACHE["nc"] = _build_nc()
    return _CACHE["nc"]


def _stat_blocks(W, mc_n, kc_n, dtype=np.float16):
    """[K, M] weight -> [MC, 128, KC, 128] blocked stationary layout:
    block[mc][p][kc][m] = W[kc*128+p, mc*128+m]."""
    W4 = np.ascontiguousarray(W, dtype=dtype).reshape(kc_n, 128, mc_n, 128)
    return np.ascontiguousarray(W4.transpose(2, 1, 0, 3))


def _featmaj(x, dtype=np.float16):
    """[T, D] -> [128, D/128, T] feature-major tile layout."""
    xT = np.ascontiguousarray(np.asarray(x, np.float32).astype(dtype)).T  # [D, T]
    d, t = xT.shape
    return np.ascontiguousarray(xT.reshape(d // 128, 128, t).transpose(1, 0, 2))


def prepare_inputs(xz, yz, wq_s, wk_s, wv_s, wo_s, wq_c, wk_c, wv_c, wo_c,
                   ffn_w1, ffn_b1, ffn_w2, ffn_b2,
                   ln0_g, ln0_b, ln1_g, ln1_b, ln2_g, ln2_b,
                   look_ahead_mask, pad_mask):
    def headcat(w):  # [H, D, HD] -> [D, H*HD]
        return np.ascontiguousarray(
            np.transpose(np.asarray(w, np.float32), (1, 0, 2)).reshape(D, D))

    shared = {
        "wq_s": _stat_blocks(headcat(wq_s), KC, KC),
        "wk_s": _stat_blocks(headcat(wk_s), KC, KC),
        "wo_s": _stat_blocks(np.asarray(wo_s, np.float32), KC, KC),
        "wq_c": _stat_blocks(headcat(wq_c), KC, KC),
        "wk_c": _stat_blocks(headcat(wk_c), KC, KC),
        "wo_c": _stat_blocks(np.asarray(wo_c, np.float32), KC, KC),
        "wv_s": np.ascontiguousarray(headcat(wv_s).astype(np.float16).reshape(KC, 128, D)),
        "wv_c": np.ascontiguousarray(headcat(wv_c).astype(np.float16).reshape(KC, 128, D)),
        "w1": _stat_blocks(np.asarray(ffn_w1, np.float32), FC, KC),
        "w2": _stat_blocks(np.asarray(ffn_w2, np.float32), KC, FC),
    }
    # additive mask, transposed to [s, t], tiled [128, SC, S]
    m = np.asarray(look_ahead_mask, np.float32)[0, 0]  # [t, s]
    mT = np.exp(np.float32(-1e9) * m.T).astype(np.float16)  # [s, t], {1,0}
    shared["maskT"] = np.ascontiguousarray(
        mT.reshape(SC, 128, S).transpose(1, 0, 2))

    def col8(v):
        return np.asarray(v, np.float32).reshape(KC, 128).T
    const_common = [np.asarray(ffn_b1, np.float32).reshape(FC, 128).T,
                    col8(ffn_b2), col8(ln0_g), col8(ln0_b), col8(ln1_g),
                    col8(ln1_b), col8(ln2_g), col8(ln2_b)]
    in_maps = []
    for c in range(B):
        im = dict(shared)
        im["yzT"] = _featmaj(np.asarray(yz, np.float32)[c])
        im["xzT"] = _featmaj(np.asarray(xz, np.float32)[c])
        pb = (np.asarray(pad_mask, np.float32)[c, 0, 0] * NEGBIG).astype(np.float32)
        im["consts"] = np.ascontiguousarray(np.concatenate(
            const_common + [pb.reshape(SC, 128).T], axis=1, dtype=np.float32))
        in_maps.append(im)
    return in_maps


def kernel(**inputs):
    from concourse.bass_utils import run_bass_kernel_spmd

    nc = _get_nc()
    in_maps = prepare_inputs(**inputs)
    res = run_bass_kernel_spmd(nc, in_maps, core_ids=list(range(B)))
    out = np.empty((B, S, D), np.float32)
    for c in range(B):
        oT = res.results[c]["outT"].astype(np.float32)  # [128, KC, S]
        out[c] = oT.transpose(1, 0, 2).reshape(D, S).T
    return out



# revision 28
# speedup vs baseline: 1.1805x; 1.1805x over previous
"""Trainium2 Bass kernel for a transformer decoder layer (self-attn + cross-attn + FFN,
3 LayerNorms). Data-parallel over batch: 8 batch elements -> 8 NeuronCores, no collectives.

Per-core dataflow (one batch element, S=512, D=1024, H=16, HD=64, DFF=4096):
  - Activations live feature-major in SBUF: X^T [D, T] as tiles [128, D/128, T].
  - Projections: out X'^T[mc] = sum_kc W[kc,mc].T @ X^T[kc]  (weights stationary).
  - Scores computed transposed: s^T[s,t] = k_h^T(.,s).T @ q_h^T  (K=HD=64).
  - Softmax without max-subtraction (scores are O(1); masked entries get -1e5 -> exp==0).
    Denominator via a ones-column prepended to V in the AV matmul (psum row 0).
  - AV: bU^T[hd,t] = V_aug[s,:].T @ exp(s^T), normalize with partition-broadcast 1/denom.
  - LayerNorm feature-major: stats over partitions via ones-vector matmuls, apply with
    partition-broadcast mean/rstd.
All host-side reshapes/transposes (inputs, weights, output) are in kernel() below.
"""
import numpy as np

B, S, D, H, HD, DFF = 8, 512, 1024, 16, 64, 4096
KC = D // 128    # 8 feature chunks
SC = S // 128    # 4 sequence chunks
FC = DFF // 128  # 32 ffn chunks
QSCALE = float(1.0 / (np.sqrt(np.float32(1024.0)) + 1e-8))
NEGBIG = -6.0e4  # fits fp16
LN_EPS = 1e-3
WSCALE = 64.0    # fp8 weight pre-scale (keeps |w| out of the denormal range)
INV_W = 1.0 / WSCALE

_CACHE = {}


def _build_nc(phases=6, look=2, pmmb=5, pavb=3, loop_n=1):
    import concourse.mybir as mybir
    import concourse.tile as tile
    from concourse import bacc

    F32 = mybir.dt.float32
    F32R = mybir.dt.float32r
    F16 = mybir.dt.float16
    F8 = mybir.dt.float8e4
    AF = mybir.ActivationFunctionType
    OP = mybir.AluOpType
    DR = mybir.MatmulPerfMode.DoubleRow

    nc = bacc.Bacc("TRN2", target_bir_lowering=False, debug=False, num_devices=8)

    # ---- DRAM I/O ----
    yzT = nc.dram_tensor("yzT", [128, KC, S], F16, kind="ExternalInput")
    yzT8_d = nc.dram_tensor("yzT8", [128, KC, S], F8, kind="ExternalInput")
    xzT8_d = nc.dram_tensor("xzT8", [128, KC, S], F8, kind="ExternalInput")
    maskT_d = nc.dram_tensor("maskT", [128, SC, S], F16, kind="ExternalInput")
    wdec = {}
    for nm in ("wq_s", "wk_s", "wq_c", "wk_c"):
        wdec[nm] = nc.dram_tensor(nm, [KC, 128, KC, 128], F8, kind="ExternalInput")
    for nm in ("wo_s", "wo_c"):
        wdec[nm] = nc.dram_tensor(nm, [KC, 128, KC, 128], F8, kind="ExternalInput")
    for nm in ("wv_s", "wv_c"):
        wdec[nm] = nc.dram_tensor(nm, [KC, 128, D], F8, kind="ExternalInput")
    w1_d = nc.dram_tensor("w1", [FC, 128, KC, 128], F16, kind="ExternalInput")
    w2_d = nc.dram_tensor("w2", [KC, 128, FC, 128], F16, kind="ExternalInput")
    consts_d = nc.dram_tensor("consts", [128, FC + KC * 7 + SC], F32,
                              kind="ExternalInput")
    outT_d = nc.dram_tensor("outT", [128, KC, S], F16, kind="ExternalOutput")

    with tile.TileContext(nc) as tc:
        with tc.tile_pool(name="persist", bufs=1) as persist:
            def _iter_body(_it=None):
                # ---- constants / persistent activations ----
                ones_t = persist.tile([128, 1], F16, tag="ones", name="ones_t")
                nc.vector.memset(ones_t[:], 1.0)
                eps_t = persist.tile([1, 1], F32, tag="eps", name="eps_t")
                nc.vector.memset(eps_t[:], LN_EPS)
                yzT_t = persist.tile([128, KC, S], F16, tag="yzT", name="yzT_t")
                nc.scalar.dma_start(out=yzT_t[:], in_=yzT[:, :, :])
                yzT8_t = persist.tile([128, KC, S], F8, tag="yzT8", name="yzT8_t")
                nc.scalar.dma_start(out=yzT8_t[:], in_=yzT8_d[:, :, :])
                consts_t = persist.tile([128, FC + KC * 7 + SC], F32, tag="consts",
                                        name="consts_t")
                nc.scalar.dma_start(out=consts_t[:], in_=consts_d[:, :])
                b1_t = consts_t[:, 0:FC]
                b2_t = consts_t[:, FC:FC + KC]
                ln_t = {}
                for i, nm in enumerate(("g0", "be0", "g1", "be1", "g2", "be2")):
                    off = FC + KC + i * KC
                    ln_t[nm] = consts_t[:, off:off + KC]
                padb_t = consts_t[:, FC + KC * 7:FC + KC * 7 + SC]
                xzT8_t = persist.tile([128, KC, S], F8, tag="xzT8", name="xzT8_t")
                o1T_t = persist.tile([128, KC, S], F16, tag="o1T", name="o1T_t")
                o1T8_t = persist.tile([128, KC, S], F8, tag="o1T8", name="o1T8_t")
                o2T_t = persist.tile([128, KC, S], F16, tag="o2T", name="o2T_t")
                resT_t = persist.tile([128, KC, S], F16, tag="resT", name="resT_t")

                # ---------------- helpers ----------------
                def emit_attention(idx, qsrc8, kv8, wq, wk, wv, wo, is_self, residT):
                    """Writes resT_t = attn_out + residT (feature-major chunks).
                    qsrc8/kv8 are fp8 activations; QKV matmuls run fp8 DoubleRow
                    (2 feature chunks per pass), weights pre-scaled by WSCALE."""
                    with tc.tile_pool(name=f"attn{idx}", bufs=1) as ap, \
                         tc.tile_pool(name=f"attn{idx}_a", bufs=4) as apool, \
                         tc.tile_pool(name=f"attn{idx}_w", bufs=2) as wpool, \
                         tc.tile_pool(name=f"attn{idx}_s", bufs=6) as spool, \
                         tc.tile_pool(name=f"attn{idx}_d", bufs=6) as dnmp, \
                         tc.tile_pool(name=f"attn{idx}_pm", bufs=pmmb, space="PSUM") as pmm, \
                         tc.tile_pool(name=f"attn{idx}_pav", bufs=pavb, space="PSUM") as pavp:
                        QT = ap.tile([128, KC, S], F16, tag="QT", name=f"QT{idx}")
                        KT = ap.tile([128, KC, S], F16, tag="KT", name=f"KT{idx}")
                        Vt = ap.tile([128, SC, H, HD + 1], F8, tag="Vt", name=f"Vt{idx}")
                        bT = ap.tile([128, KC, S], F8, tag="bT", name=f"bT{idx}")

                        # K and V first (kv8-dependent; overlaps with the LN
                        # producing qsrc8 for cross-attention), then Q.
                        # V: normal layout [s, (h hd)] with ones column at 64
                        for j in range(SC):
                            nc.vector.memset(Vt[:, j, :, HD:HD + 1], 1.0)
                        wvt = wpool.tile([128, KC, D], F8, tag="wmv",
                                         name=f"wv{idx}")
                        nc.sync.dma_start(out=wvt[:], in_=wv[:, :, :].rearrange(
                            "k p m -> p k m"))
                        for nh in range(2):
                            for j in range(SC):
                                pv = pmm.tile([128, S], F32, tag="pmm",
                                              name=f"pv{idx}_{nh}_{j}")
                                for kc in range(0, KC, 2):
                                    nc.tensor.matmul(
                                        pv[:],
                                        kv8[:, kc:kc + 2, 128 * j:128 * (j + 1)],
                                        wvt[:, kc:kc + 2, 512 * nh:512 * (nh + 1)],
                                        start=(kc == 0), stop=(kc == KC - 2),
                                        perf_mode=DR)
                                # psum [128, 512] -> V[:, j, 8nh:8nh+8, 0:64]
                                nc.scalar.activation(
                                    out=Vt[:, j, 8 * nh:8 * (nh + 1), 0:HD],
                                    in_=pv[:], func=AF.Copy, scale=INV_W)
                        for which, wsrc, src, dst in (("k", wk, kv8, KT),
                                                      ("q", wq, qsrc8, QT)):
                            wt = wpool.tile([128, KC, KC, 128], F8, tag="wst",
                                            name=f"w{which}{idx}")
                            nc.sync.dma_start(out=wt[:], in_=wsrc[:, :, :, :].rearrange(
                                "mc p kc m -> p mc kc m"))
                            for mc in range(KC):
                                pq = pmm.tile([128, S], F32, tag="pmm",
                                              name=f"p{which}{idx}_{mc}")
                                for kc in range(0, KC, 2):
                                    nc.tensor.matmul(pq[:], wt[:, mc, kc:kc + 2, :],
                                                     src[:, kc:kc + 2, :],
                                                     start=(kc == 0), stop=(kc == KC - 2),
                                                     perf_mode=DR)
                                # PSUM->SBUF rescale on DVE (Act is the bottleneck)
                                sc_ = QSCALE * INV_W if which == "q" else INV_W
                                nc.vector.tensor_scalar_mul(out=dst[:, mc, :],
                                                            in0=pq[:], scalar1=sc_)

                        # per-head scores + AV, software-pipelined one head ahead
                        # so the in-order PE never waits on the exp/mask chain.
                        aTs = {}
                        if is_self:
                            # causal: aT[:, j, t<128j] is identically 0. The 4
                            # cycling aT bufs are pre-zeroed once; exp never
                            # writes there, so zeros persist across head reuse.
                            for bi in range(4):
                                aT = apool.tile([128, SC, S], F8, tag="aT",
                                                name=f"aTz{idx}_{bi}")
                                nc.vector.memset(aT[:, 1:SC, 0:3 * 128], 0.0)

                        def emit_scores(h):
                            mc, off = h // 2, 64 * (h % 2)
                            aT = apool.tile([128, SC, S], F8, tag="aT",
                                            name=f"aT{idx}_{h}")
                            for j in range(SC):
                                t0 = 128 * j if is_self else 0
                                pst = pmm.tile([128, S], F32, tag="pmm",
                                               name=f"ps{idx}_{h}_{j}")
                                nc.tensor.matmul(
                                    pst[:, t0:S],
                                    KT[off:off + 64, mc, 128 * j:128 * (j + 1)],
                                    QT[off:off + 64, mc, t0:S],
                                    start=True, stop=True)
                                if is_self:
                                    # additive causal mask, diagonal block only
                                    nc.vector.tensor_add(
                                        out=pst[:, t0:t0 + 128], in0=pst[:, t0:t0 + 128],
                                        in1=maskT_t[:, j, t0:t0 + 128])
                                    nc.scalar.activation(out=aT[:, j, t0:S],
                                                         in_=pst[:, t0:S],
                                                         func=AF.Exp)
                                else:
                                    nc.scalar.activation(out=aT[:, j, :], in_=pst[:],
                                                         func=AF.Exp,
                                                         bias=padb_t[:, j:j + 1])
                            aTs[h] = aT

                        def emit_av(h):
                            mc, off = h // 2, 64 * (h % 2)
                            aT = aTs.pop(h)
                            pav = pavp.tile([HD + 1, S], F32, tag="pav",
                                            name=f"pav{idx}_{h}")
                            for jj in range(0, SC, 2):
                                t0 = 128 * jj if is_self else 0
                                nc.tensor.matmul(pav[:, t0:S], Vt[:, jj:jj + 2, h, :],
                                                 aT[:, jj:jj + 2, t0:S],
                                                 start=(jj == 0), stop=(jj == SC - 2),
                                                 perf_mode=DR, skip_group_check=True)
                            dnm = dnmp.tile([1, S], F16, tag="dnm", name=f"dnm{idx}_{h}")
                            nc.vector.tensor_scalar_mul(out=dnm[:],
                                                        in0=pav[HD:HD + 1, :],
                                                        scalar1=1.0)
                            rb = spool.tile([64, S], F16, tag="rb", name=f"rb{idx}_{h}")
                            nc.gpsimd.partition_broadcast(rb[:], dnm[:])
                            with nc.allow_low_precision(reason="softmax denom recip fp16"):
                                nc.vector.reciprocal(out=rb[:], in_=rb[:])
                            nc.vector.tensor_mul(out=bT[off:off + 64, mc, :],
                                                 in0=pav[0:HD, :], in1=rb[:])

                        for h in range(look):
                            emit_scores(h)
                        for h in range(H):
                            if h + look < H:
                                emit_scores(h + look)
                            emit_av(h)

                        # output projection (fp8 DoubleRow) + residual add
                        wt = wpool.tile([128, KC, KC, 128], F8, tag="wst",
                                        name=f"wo{idx}")
                        nc.sync.dma_start(out=wt[:], in_=wo[:, :, :, :].rearrange(
                            "mc p kc m -> p mc kc m"))
                        for mc in range(KC):
                            po = pmm.tile([128, S], F32, tag="pmm", name=f"po{idx}_{mc}")
                            for kc in range(0, KC, 2):
                                nc.tensor.matmul(po[:], wt[:, mc, kc:kc + 2, :],
                                                 bT[:, kc:kc + 2, :],
                                                 start=(kc == 0), stop=(kc == KC - 2),
                                                 perf_mode=DR)
                            nc.vector.scalar_tensor_tensor(
                                out=resT_t[:, mc, :], in0=po[:], scalar=INV_W,
                                in1=residT[:, mc, :], op0=OP.mult, op1=OP.add)

                def emit_ln(idx, g_ap, b_ap, outT, outT8=None):
                    """LayerNorm over features of resT_t -> outT ([128, KC, S] tile or
                    None to stream to DRAM output); outT8 gets an fp8 copy."""
                    with tc.tile_pool(name=f"ln{idx}_ps", bufs=4, space="PSUM") as pstat, \
                         tc.tile_pool(name=f"ln{idx}_st", bufs=3) as stage, \
                         tc.tile_pool(name=f"ln{idx}_x", bufs=KC) as lnx, \
                         tc.tile_pool(name=f"ln{idx}_sm", bufs=1) as lnsm:
                        # two parallel accumulation chains per stat to cut latency
                        psms = [pstat.tile([1, S], F32, tag="psm", name=f"psm{idx}_{i}")
                                for i in range(2)]
                        psss = [pstat.tile([1, S], F32, tag="psm", name=f"pss{idx}_{i}")
                                for i in range(2)]
                        xsqs = []
                        for kc in range(KC):
                            i, last = kc % 2, kc >= KC - 2
                            nc.tensor.matmul(psms[i][:], ones_t[:], resT_t[:, kc, :],
                                             start=(kc < 2), stop=last)
                            xsq = lnx.tile([128, S], F16, tag="xsq",
                                           name=f"xsq{idx}_{kc}")
                            eng = nc.gpsimd if kc % 2 else nc.scalar
                            if eng is nc.scalar:
                                eng.activation(out=xsq[:], in_=resT_t[:, kc, :],
                                               func=AF.Square)
                            else:
                                eng.tensor_mul(out=xsq[:], in0=resT_t[:, kc, :],
                                               in1=resT_t[:, kc, :])
                            xsqs.append(xsq)
                        for kc in range(KC):
                            i, last = kc % 2, kc >= KC - 2
                            # xsq tiles cycle through 2 stage bufs; re-derive handle
                            nc.tensor.matmul(psss[i][:], ones_t[:],
                                             xsqs[kc][:],
                                             start=(kc < 2), stop=last)
                        m1 = lnsm.tile([1, S], F32, tag="m1", name=f"m1_{idx}")
                        nc.scalar.activation(out=m1[:], in_=psms[1][:], func=AF.Copy)
                        mrow = lnsm.tile([1, S], F32, tag="mrow", name=f"mrow{idx}")
                        nc.vector.tensor_add(out=mrow[:], in0=psms[0][:], in1=m1[:])
                        e1 = lnsm.tile([1, S], F32, tag="e1", name=f"e1_{idx}")
                        nc.scalar.activation(out=e1[:], in_=psss[1][:], func=AF.Copy)
                        erow = lnsm.tile([1, S], F32, tag="erow", name=f"erow{idx}")
                        nc.vector.tensor_add(out=erow[:], in0=psss[0][:], in1=e1[:])
                        nc.scalar.activation(out=mrow[:], in_=mrow[:], func=AF.Copy,
                                             scale=1.0 / D)
                        nc.scalar.activation(out=erow[:], in_=erow[:], func=AF.Copy,
                                             scale=1.0 / D)
                        vrow = lnsm.tile([1, S], F32, tag="vrow", name=f"vrow{idx}")
                        nc.vector.tensor_mul(out=vrow[:], in0=mrow[:], in1=mrow[:])
                        nc.vector.tensor_sub(out=vrow[:], in0=erow[:], in1=vrow[:])
                        nc.scalar.activation(out=vrow[:], in_=vrow[:], func=AF.Sqrt,
                                             bias=eps_t[:])
                        nc.vector.reciprocal(out=vrow[:], in_=vrow[:])
                        mrow16 = lnsm.tile([1, S], F16, tag="mrow16", name=f"mrow16{idx}")
                        nc.scalar.activation(out=mrow16[:], in_=mrow[:], func=AF.Copy)
                        vrow16 = lnsm.tile([1, S], F16, tag="vrow16", name=f"vrow16{idx}")
                        nc.scalar.activation(out=vrow16[:], in_=vrow[:], func=AF.Copy)
                        meanB = stage.tile([128, S], F16, tag="lnB", name=f"meanB{idx}")
                        nc.gpsimd.partition_broadcast(meanB[:], mrow16[:])
                        rstdB = stage.tile([128, S], F16, tag="lnB", name=f"rstdB{idx}")
                        nc.gpsimd.partition_broadcast(rstdB[:], vrow16[:])
                        for kc in range(KC):
                            dte = stage.tile([128, S], F16, tag="lnd", name=f"lnd{idx}_{kc}")
                            eng = nc.vector if kc % 4 != 3 else nc.gpsimd
                            eng.tensor_sub(out=dte[:], in0=resT_t[:, kc, :],
                                           in1=meanB[:])
                            nc.vector.scalar_tensor_tensor(
                                out=dte[:], in0=dte[:], scalar=g_ap[:, kc:kc + 1],
                                in1=rstdB[:], op0=OP.mult, op1=OP.mult)
                            if outT is not None:
                                nc.scalar.activation(out=outT[:, kc, :], in_=dte[:],
                                                     func=AF.Identity,
                                                     bias=b_ap[:, kc:kc + 1])
                                if outT8 is not None:
                                    nc.scalar.activation(out=outT8[:, kc, :],
                                                         in_=dte[:],
                                                         func=AF.Identity,
                                                         bias=b_ap[:, kc:kc + 1])
                            else:
                                ote = stage.tile([128, S], F16, tag="lno",
                                                 name=f"lno{idx}_{kc}")
                                nc.scalar.activation(out=ote[:], in_=dte[:],
                                                     func=AF.Identity,
                                                     bias=b_ap[:, kc:kc + 1])
                                nc.scalar.dma_start(out=outT_d[:, kc, :], in_=ote[:])

                def emit_ffn():
                    """resT_t = relu(o2T @ w1 + b1) @ w2 + b2 + o2T  (fp8 DoubleRow)"""
                    with tc.tile_pool(name="ffn", bufs=1) as fp, \
                         tc.tile_pool(name="ffn_w", bufs=3) as fw, \
                         tc.tile_pool(name="ffn_w2", bufs=2) as fw2, \
                         tc.tile_pool(name="ffn_st", bufs=3) as fst, \
                         tc.tile_pool(name="ffn_ps", bufs=4, space="PSUM") as pf:
                        hT = fp.tile([128, FC, S], F16, tag="hT", name="hT")
                        for g in range(FC // 4):
                            wt = fw.tile([128, 4, KC, 128], F16, tag="w1t", name=f"w1_{g}")
                            nc.sync.dma_start(out=wt[:], in_=w1_d[4 * g:4 * (g + 1), :, :, :]
                                              .rearrange("mc p kc m -> p mc kc m"))
                            for mi in range(4):
                                mc = 4 * g + mi
                                ph = pf.tile([128, S], F32, tag="pf", name=f"ph{mc}")
                                for kc in range(KC):
                                    nc.tensor.matmul(ph[:], wt[:, mi, kc, :],
                                                     o2T_t[:, kc, :],
                                                     start=(kc == 0), stop=(kc == KC - 1))
                                nc.scalar.activation(out=hT[:, mc, :], in_=ph[:],
                                                     func=AF.Relu, bias=b1_t[:, mc:mc + 1])
                        for mc in range(KC):
                            wt = fw2.tile([128, FC, 128], F16, tag="w2t", name=f"w2_{mc}")
                            nc.sync.dma_start(out=wt[:], in_=w2_d[mc, :, :, :])
                            po = pf.tile([128, S], F32, tag="pf", name=f"po2_{mc}")
                            for kc in range(FC):
                                nc.tensor.matmul(po[:], wt[:, kc, :], hT[:, kc, :],
                                                 start=(kc == 0), stop=(kc == FC - 1))
                            # resT = (po + b2) + o2T
                            nc.vector.scalar_tensor_tensor(
                                out=resT_t[:, mc, :], in0=po[:], scalar=b2_t[:, mc:mc + 1],
                                in1=o2T_t[:, mc, :], op0=OP.add, op1=OP.add)

                # ---------------- main flow ----------------
                with tc.tile_pool(name="maskp", bufs=1) as maskp:
                    maskT_t = maskp.tile([128, SC, S], F16, tag="maskT", name="maskT_t")
                    nc.scalar.dma_start(out=maskT_t[:], in_=maskT_d[:, :, :])

                    if phases >= 1:
                        emit_attention(0, yzT8_t, yzT8_t, wdec["wq_s"], wdec["wk_s"],
                                       wdec["wv_s"], wdec["wo_s"], True, yzT_t)
                    if phases >= 2:
                        emit_ln(0, ln_t["g0"], ln_t["be0"], o1T_t, o1T8_t)
                    nc.scalar.dma_start(out=xzT8_t[:], in_=xzT8_d[:, :, :])
                    if phases >= 3:
                        emit_attention(1, o1T8_t, xzT8_t, wdec["wq_c"], wdec["wk_c"],
                                       wdec["wv_c"], wdec["wo_c"], False, o1T_t)
                    if phases >= 4:
                        emit_ln(1, ln_t["g1"], ln_t["be1"], o2T_t)
                if phases >= 5:
                    emit_ffn()
                if phases >= 6:
                    emit_ln(2, ln_t["g2"], ln_t["be2"], None)


            if loop_n == 1:
                _iter_body()
            else:
                with tc.For_i(0, loop_n, 1) as _it:
                    _iter_body(_it)

    nc.compile()
    return nc


def _get_nc():
    if "nc" not in _CACHE:
        _CACHE["nc"] = _build_nc()
    return _CACHE["nc"]


def _stat_blocks(W, mc_n, kc_n, dtype=np.float16):
    """[K, M] weight -> [MC, 128, KC, 128] blocked stationary layout:
    block[mc][p][kc][m] = W[kc*128+p, mc*128+m]."""
    W4 = np.ascontiguousarray(W, dtype=dtype).reshape(kc_n, 128, mc_n, 128)
    return np.ascontiguousarray(W4.transpose(2, 1, 0, 3))


def _featmaj(x, dtype=np.float16):
    """[T, D] -> [128, D/128, T] feature-major tile layout."""
    xT = np.ascontiguousarray(np.asarray(x, np.float32).astype(dtype)).T  # [D, T]
    d, t = xT.shape
    return np.ascontiguousarray(xT.reshape(d // 128, 128, t).transpose(1, 0, 2))


def prepare_inputs(xz, yz, wq_s, wk_s, wv_s, wo_s, wq_c, wk_c, wv_c, wo_c,
                   ffn_w1, ffn_b1, ffn_w2, ffn_b2,
                   ln0_g, ln0_b, ln1_g, ln1_b, ln2_g, ln2_b,
                   look_ahead_mask, pad_mask):
    import ml_dtypes
    F8NP = ml_dtypes.float8_e4m3

    def headcat(w):  # [H, D, HD] -> [D, H*HD]
        return np.ascontiguousarray(
            np.transpose(np.asarray(w, np.float32), (1, 0, 2)).reshape(D, D))

    shared = {
        "wq_s": _stat_blocks(headcat(wq_s) * WSCALE, KC, KC, F8NP),
        "wk_s": _stat_blocks(headcat(wk_s) * WSCALE, KC, KC, F8NP),
        "wo_s": _stat_blocks(np.asarray(wo_s, np.float32) * WSCALE, KC, KC, F8NP),
        "wq_c": _stat_blocks(headcat(wq_c) * WSCALE, KC, KC, F8NP),
        "wk_c": _stat_blocks(headcat(wk_c) * WSCALE, KC, KC, F8NP),
        "wo_c": _stat_blocks(np.asarray(wo_c, np.float32) * WSCALE, KC, KC, F8NP),
        "wv_s": np.ascontiguousarray(
            (headcat(wv_s) * WSCALE).astype(F8NP).reshape(KC, 128, D)),
        "wv_c": np.ascontiguousarray(
            (headcat(wv_c) * WSCALE).astype(F8NP).reshape(KC, 128, D)),
        "w1": _stat_blocks(np.asarray(ffn_w1, np.float32), FC, KC),
        "w2": _stat_blocks(np.asarray(ffn_w2, np.float32), KC, FC),
    }
    # additive mask, transposed to [s, t], tiled [128, SC, S]: 0 or NEGBIG
    m = np.asarray(look_ahead_mask, np.float32)[0, 0]  # [t, s]
    mT = (m.T * np.float32(NEGBIG)).astype(np.float16)  # [s, t], {0, NEGBIG}
    shared["maskT"] = np.ascontiguousarray(
        mT.reshape(SC, 128, S).transpose(1, 0, 2))

    def col8(v):
        return np.asarray(v, np.float32).reshape(KC, 128).T
    const_common = [np.asarray(ffn_b1, np.float32).reshape(FC, 128).T,
                    col8(ffn_b2), col8(ln0_g), col8(ln0_b), col8(ln1_g),
                    col8(ln1_b), col8(ln2_g), col8(ln2_b)]
    in_maps = []
    for c in range(B):
        im = dict(shared)
        im["yzT"] = _featmaj(np.asarray(yz, np.float32)[c])
        im["yzT8"] = _featmaj(np.asarray(yz, np.float32)[c], F8NP)
        im["xzT8"] = _featmaj(np.asarray(xz, np.float32)[c], F8NP)
        pb = (np.asarray(pad_mask, np.float32)[c, 0, 0] * NEGBIG).astype(np.float32)
        im["consts"] = np.ascontiguousarray(np.concatenate(
            const_common + [pb.reshape(SC, 128).T], axis=1, dtype=np.float32))
        in_maps.append(im)
    return in_maps


def kernel(**inputs):
    from concourse.bass_utils import run_bass_kernel_spmd

    nc = _get_nc()
    in_maps = prepare_inputs(**inputs)
    res = run_bass_kernel_spmd(nc, in_maps, core_ids=list(range(B)))
    out = np.empty((B, S, D), np.float32)
    for c in range(B):
        oT = res.results[c]["outT"].astype(np.float32)  # [128, KC, S]
        out[c] = oT.transpose(1, 0, 2).reshape(D, S).T
    return out



# revision 41
# speedup vs baseline: 1.4583x; 1.2353x over previous
"""Trainium2 Bass kernel for a transformer decoder layer (self-attn + cross-attn + FFN,
3 LayerNorms). Data-parallel over batch: 8 batch elements -> 8 NeuronCores, no collectives.

Per-core dataflow (one batch element, S=512, D=1024, H=16, HD=64, DFF=4096):
  - Activations live feature-major in SBUF: X^T [D, T] as tiles [128, D/128, T].
  - Projections: out X'^T[mc] = sum_kc W[kc,mc].T @ X^T[kc]  (weights stationary).
  - Scores computed transposed: s^T[s,t] = k_h^T(.,s).T @ q_h^T  (K=HD=64).
  - Softmax without max-subtraction (scores are O(1); masked entries get -1e5 -> exp==0).
    Denominator via a ones-column prepended to V in the AV matmul (psum row 0).
  - AV: bU^T[hd,t] = V_aug[s,:].T @ exp(s^T), normalize with partition-broadcast 1/denom.
  - LayerNorm feature-major: stats over partitions via ones-vector matmuls, apply with
    partition-broadcast mean/rstd.
All host-side reshapes/transposes (inputs, weights, output) are in kernel() below.
"""
import numpy as np

B, S, D, H, HD, DFF = 8, 512, 1024, 16, 64, 4096
KC = D // 128    # 8 feature chunks
SC = S // 128    # 4 sequence chunks
FC = DFF // 128  # 32 ffn chunks
QSCALE = float(1.0 / (np.sqrt(np.float32(1024.0)) + 1e-8))
NEGBIG = -6.0e4  # fits fp16
LN_EPS = 1e-3
WSCALE = 64.0    # fp8 weight pre-scale (keeps |w| out of the denormal range)
INV_W = 1.0 / WSCALE

_CACHE = {}


def _build_nc(phases=6, look=2, pmmb=5, pavb=3, loop_n=1):
    import concourse.mybir as mybir
    import concourse.tile as tile
    from concourse import bacc

    F32 = mybir.dt.float32
    F32R = mybir.dt.float32r
    F16 = mybir.dt.float16
    F8 = mybir.dt.float8e4
    AF = mybir.ActivationFunctionType
    OP = mybir.AluOpType
    DR = mybir.MatmulPerfMode.DoubleRow

    nc = bacc.Bacc("TRN2", target_bir_lowering=False, debug=False, num_devices=8)

    # ---- DRAM I/O ----
    yzT = nc.dram_tensor("yzT", [128, KC, S], F16, kind="ExternalInput")
    yzT8_d = nc.dram_tensor("yzT8", [128, KC, S], F8, kind="ExternalInput")
    xzT8_d = nc.dram_tensor("xzT8", [128, KC, S], F8, kind="ExternalInput")
    maskT_d = nc.dram_tensor("maskT", [128, SC, S], F16, kind="ExternalInput")
    wdec = {}
    for nm in ("wq_s", "wk_s", "wq_c", "wk_c"):
        wdec[nm] = nc.dram_tensor(nm, [KC, 128, KC, 128], F8, kind="ExternalInput")
    for nm in ("wo_s", "wo_c"):
        wdec[nm] = nc.dram_tensor(nm, [KC, 128, KC, 128], F16, kind="ExternalInput")
    for nm in ("wv_s", "wv_c"):
        wdec[nm] = nc.dram_tensor(nm, [KC, 128, D], F8, kind="ExternalInput")
    w1_d = nc.dram_tensor("w1", [FC, 128, KC, 128], F16, kind="ExternalInput")
    w2_d = nc.dram_tensor("w2", [KC, 128, FC, 128], F16, kind="ExternalInput")
    consts_d = nc.dram_tensor("consts", [128, FC + KC * 7 + SC], F32,
                              kind="ExternalInput")
    outT_d = nc.dram_tensor("outT", [128, KC, S], F16, kind="ExternalOutput")

    with tile.TileContext(nc) as tc:
        with tc.tile_pool(name="persist", bufs=1) as persist:
            def _iter_body(_it=None):
                # ---- constants / persistent activations ----
                ones_t = persist.tile([128, 1], F16, tag="ones", name="ones_t")
                nc.vector.memset(ones_t[:], 1.0)
                eps_t = persist.tile([1, 1], F32, tag="eps", name="eps_t")
                nc.vector.memset(eps_t[:], LN_EPS)
                yzT_t = persist.tile([128, KC, S], F16, tag="yzT", name="yzT_t")
                nc.scalar.dma_start(out=yzT_t[:], in_=yzT[:, :, :])
                yzT8_t = persist.tile([128, KC, S], F8, tag="yzT8", name="yzT8_t")
                nc.scalar.dma_start(out=yzT8_t[:], in_=yzT8_d[:, :, :])
                consts_t = persist.tile([128, FC + KC * 7 + SC], F32, tag="consts",
                                        name="consts_t")
                nc.scalar.dma_start(out=consts_t[:], in_=consts_d[:, :])
                b1_t = consts_t[:, 0:FC]
                b2_t = consts_t[:, FC:FC + KC]
                ln_t = {}
                for i, nm in enumerate(("g0", "be0", "g1", "be1", "g2", "be2")):
                    off = FC + KC + i * KC
                    ln_t[nm] = consts_t[:, off:off + KC]
                padb_t = consts_t[:, FC + KC * 7:FC + KC * 7 + SC]
                xzT8_t = persist.tile([128, KC, S], F8, tag="xzT8", name="xzT8_t")
                o1T_t = persist.tile([128, KC, S], F16, tag="o1T", name="o1T_t")
                o1T8_t = persist.tile([128, KC, S], F8, tag="o1T8", name="o1T8_t")
                o2T_t = persist.tile([128, KC, S], F16, tag="o2T", name="o2T_t")
                resT_t = persist.tile([128, KC, S], F16, tag="resT", name="resT_t")

                # ---------------- helpers ----------------
                def make_kv_thunks(idx, kv8, wk, wv, KT, Vt, wpool):
                    """K/V projections for attention idx as thunks taking the
                    active PSUM pool — lets cross-attn K/V interleave into the
                    self-attn head loop (fills PE idle gaps)."""
                    wvt = wpool.tile([128, KC, D], F8, tag="wmv", name=f"wv{idx}")
                    wkt = wpool.tile([128, KC, KC, 128], F8, tag="wst",
                                     name=f"wk{idx}")

                    def pre(_pmm):
                        nc.sync.dma_start(out=wvt[:], in_=wv[:, :, :].rearrange(
                            "k p m -> p k m"))
                        nc.sync.dma_start(out=wkt[:], in_=wk[:, :, :, :].rearrange(
                            "mc p kc m -> p mc kc m"))
                        for j in range(SC):
                            nc.vector.memset(Vt[:, j, :, HD:HD + 1], 1.0)

                    def mkv(nh, j):
                        def t(pmm):
                            pv = pmm.tile([128, S], F32, tag="pmm",
                                          name=f"pv{idx}_{nh}_{j}")
                            for kc in range(0, KC, 2):
                                nc.tensor.matmul(
                                    pv[:],
                                    kv8[:, kc:kc + 2, 128 * j:128 * (j + 1)],
                                    wvt[:, kc:kc + 2, 512 * nh:512 * (nh + 1)],
                                    start=(kc == 0), stop=(kc == KC - 2),
                                    perf_mode=DR)
                            nc.scalar.activation(
                                out=Vt[:, j, 8 * nh:8 * (nh + 1), 0:HD],
                                in_=pv[:], func=AF.Copy, scale=INV_W)
                        return t

                    def mkk(mc):
                        def t(pmm):
                            pq = pmm.tile([128, S], F32, tag="pmm",
                                          name=f"pk{idx}_{mc}")
                            for kc in range(0, KC, 2):
                                nc.tensor.matmul(pq[:], wkt[:, mc, kc:kc + 2, :],
                                                 kv8[:, kc:kc + 2, :],
                                                 start=(kc == 0), stop=(kc == KC - 2),
                                                 perf_mode=DR)
                            nc.scalar.activation(out=KT[:, mc, :], in_=pq[:],
                                                 func=AF.Copy, scale=INV_W)
                        return t

                    return [pre] + [mkv(nh, j) for nh in range(2)
                                    for j in range(SC)] + \
                           [mkk(mc) for mc in range(KC)]

                def emit_attention(idx, qsrc8, wq, wo, is_self, residT,
                                   KT, Vt, kv_thunks, extra, wpool):
                    """Writes resT_t = attn_out + residT. kv_thunks: this
                    attention's own K/V work (run up front); extra: deque of
                    foreign thunks drained one per head during the loop."""
                    with tc.tile_pool(name=f"attn{idx}", bufs=1) as ap, \
                         tc.tile_pool(name=f"attn{idx}_a", bufs=4) as apool, \
                         tc.tile_pool(name=f"attn{idx}_s", bufs=6) as spool, \
                         tc.tile_pool(name=f"attn{idx}_d", bufs=6) as dnmp, \
                         tc.tile_pool(name=f"attn{idx}_pm", bufs=pmmb, space="PSUM") as pmm, \
                         tc.tile_pool(name=f"attn{idx}_pav", bufs=pavb, space="PSUM") as pavp:
                        QT = ap.tile([128, KC, S], F16, tag=f"QT{idx}",
                                     name=f"QT{idx}")
                        bT = ap.tile([128, KC, S], F16, tag=f"bT{idx}",
                                     name=f"bT{idx}")

                        for t in kv_thunks:
                            t(pmm)
                        if extra:
                            extra.popleft()(pmm)  # foreign weight DMAs early
                        wt = wpool.tile([128, KC, KC, 128], F8, tag="wst",
                                        name=f"wq{idx}")
                        nc.sync.dma_start(out=wt[:], in_=wq[:, :, :, :].rearrange(
                            "mc p kc m -> p mc kc m"))
                        for mc in range(KC):
                            pq = pmm.tile([128, S], F32, tag="pmm",
                                          name=f"pq{idx}_{mc}")
                            for kc in range(0, KC, 2):
                                nc.tensor.matmul(pq[:], wt[:, mc, kc:kc + 2, :],
                                                 qsrc8[:, kc:kc + 2, :],
                                                 start=(kc == 0), stop=(kc == KC - 2),
                                                 perf_mode=DR)
                            nc.scalar.activation(out=QT[:, mc, :], in_=pq[:],
                                                 func=AF.Copy, scale=QSCALE * INV_W)

                        # per-head scores + AV, software-pipelined one head ahead
                        # so the in-order PE never waits on the exp/mask chain.
                        aTs = {}
                        if is_self:
                            # causal: aT[:, j, t<128j] is identically 0. The 4
                            # cycling aT bufs are pre-zeroed once; exp never
                            # writes there, so zeros persist across head reuse.
                            for bi in range(4):
                                aT = apool.tile([128, SC, S], F8, tag="aT",
                                                name=f"aTz{idx}_{bi}")
                                nc.vector.memset(aT[:, 1:SC, 0:3 * 128], 0.0)

                        def emit_scores(h):
                            mc, off = h // 2, 64 * (h % 2)
                            aT = apool.tile([128, SC, S], F8, tag="aT",
                                            name=f"aT{idx}_{h}")
                            for j in range(SC):
                                t0 = 128 * j if is_self else 0
                                pst = pmm.tile([128, S], F32, tag="pmm",
                                               name=f"ps{idx}_{h}_{j}")
                                nc.tensor.matmul(
                                    pst[:, t0:S],
                                    KT[off:off + 64, mc, 128 * j:128 * (j + 1)],
                                    QT[off:off + 64, mc, t0:S],
                                    start=True, stop=True)
                                if is_self:
                                    # additive causal mask, diagonal block only
                                    nc.vector.tensor_add(
                                        out=pst[:, t0:t0 + 128], in0=pst[:, t0:t0 + 128],
                                        in1=maskT_t[:, j, t0:t0 + 128])
                                    nc.scalar.activation(out=aT[:, j, t0:S],
                                                         in_=pst[:, t0:S],
                                                         func=AF.Exp)
                                else:
                                    nc.scalar.activation(out=aT[:, j, :], in_=pst[:],
                                                         func=AF.Exp,
                                                         bias=padb_t[:, j:j + 1])
                            aTs[h] = aT

                        def emit_av(h):
                            mc, off = h // 2, 64 * (h % 2)
                            aT = aTs.pop(h)
                            pav = pavp.tile([HD + 1, S], F32, tag="pav",
                                            name=f"pav{idx}_{h}")
                            for jj in range(0, SC, 2):
                                t0 = 128 * jj if is_self else 0
                                nc.tensor.matmul(pav[:, t0:S], Vt[:, jj:jj + 2, h, :],
                                                 aT[:, jj:jj + 2, t0:S],
                                                 start=(jj == 0), stop=(jj == SC - 2),
                                                 perf_mode=DR, skip_group_check=True)
                            dnm = dnmp.tile([1, S], F16, tag="dnm", name=f"dnm{idx}_{h}")
                            nc.scalar.activation(out=dnm[:], in_=pav[HD:HD + 1, :],
                                                 func=AF.Copy)
                            rb = spool.tile([64, S], F16, tag="rb", name=f"rb{idx}_{h}")
                            nc.gpsimd.partition_broadcast(rb[:], dnm[:])
                            with nc.allow_low_precision(reason="softmax denom recip fp16"):
                                nc.vector.reciprocal(out=rb[:], in_=rb[:])
                            nc.vector.tensor_mul(out=bT[off:off + 64, mc, :],
                                                 in0=pav[0:HD, :], in1=rb[:])

                        for h in range(look):
                            emit_scores(h)
                        for h in range(H):
                            if h + look < H:
                                emit_scores(h + look)
                            if extra:
                                extra.popleft()(pmm)
                            emit_av(h)
                        while extra:
                            extra.popleft()(pmm)

                        # output projection + residual add
                        wt = wpool.tile([128, KC, KC, 128], F16, tag="wstO",
                                        name=f"wo{idx}")
                        nc.sync.dma_start(out=wt[:], in_=wo[:, :, :, :].rearrange(
                            "mc p kc m -> p mc kc m"))
                        for mc in range(KC):
                            po = pmm.tile([128, S], F32, tag="pmm", name=f"po{idx}_{mc}")
                            for kc in range(KC):
                                nc.tensor.matmul(po[:], wt[:, mc, kc, :], bT[:, kc, :],
                                                 start=(kc == 0), stop=(kc == KC - 1))
                            nc.vector.tensor_add(out=resT_t[:, mc, :], in0=po[:],
                                                 in1=residT[:, mc, :])

                def emit_ln(idx, g_ap, b_ap, outT, outT8=None):
                    """LayerNorm over features of resT_t -> outT ([128, KC, S] tile or
                    None to stream to DRAM output); outT8 gets an fp8 copy."""
                    with tc.tile_pool(name=f"ln{idx}_ps", bufs=4, space="PSUM") as pstat, \
                         tc.tile_pool(name=f"ln{idx}_st", bufs=3) as stage, \
                         tc.tile_pool(name=f"ln{idx}_x", bufs=KC) as lnx, \
                         tc.tile_pool(name=f"ln{idx}_sm", bufs=1) as lnsm:
                        # two parallel accumulation chains per stat to cut latency
                        psms = [pstat.tile([1, S], F32, tag="psm", name=f"psm{idx}_{i}")
                                for i in range(2)]
                        psss = [pstat.tile([1, S], F32, tag="psm", name=f"pss{idx}_{i}")
                                for i in range(2)]
                        xsqs = []
                        for kc in range(KC):
                            i, last = kc % 2, kc >= KC - 2
                            nc.tensor.matmul(psms[i][:], ones_t[:], resT_t[:, kc, :],
                                             start=(kc < 2), stop=last)
                            xsq = lnx.tile([128, S], F16, tag="xsq",
                                           name=f"xsq{idx}_{kc}")
                            eng = nc.gpsimd if kc % 2 else nc.scalar
                            if eng is nc.scalar:
                                eng.activation(out=xsq[:], in_=resT_t[:, kc, :],
                                               func=AF.Square)
                            else:
                                eng.tensor_mul(out=xsq[:], in0=resT_t[:, kc, :],
                                               in1=resT_t[:, kc, :])
                            xsqs.append(xsq)
                        for kc in range(KC):
                            i, last = kc % 2, kc >= KC - 2
                            # xsq tiles cycle through 2 stage bufs; re-derive handle
                            nc.tensor.matmul(psss[i][:], ones_t[:],
                                             xsqs[kc][:],
                                             start=(kc < 2), stop=last)
                        m1 = lnsm.tile([1, S], F32, tag="m1", name=f"m1_{idx}")
                        nc.scalar.activation(out=m1[:], in_=psms[1][:], func=AF.Copy)
                        mrow = lnsm.tile([1, S], F32, tag="mrow", name=f"mrow{idx}")
                        nc.vector.tensor_add(out=mrow[:], in0=psms[0][:], in1=m1[:])
                        e1 = lnsm.tile([1, S], F32, tag="e1", name=f"e1_{idx}")
                        nc.scalar.activation(out=e1[:], in_=psss[1][:], func=AF.Copy)
                        erow = lnsm.tile([1, S], F32, tag="erow", name=f"erow{idx}")
                        nc.vector.tensor_add(out=erow[:], in0=psss[0][:], in1=e1[:])
                        nc.scalar.activation(out=mrow[:], in_=mrow[:], func=AF.Copy,
                                             scale=1.0 / D)
                        nc.scalar.activation(out=erow[:], in_=erow[:], func=AF.Copy,
                                             scale=1.0 / D)
                        vrow = lnsm.tile([1, S], F32, tag="vrow", name=f"vrow{idx}")
                        nc.vector.tensor_mul(out=vrow[:], in0=mrow[:], in1=mrow[:])
                        nc.vector.tensor_sub(out=vrow[:], in0=erow[:], in1=vrow[:])
                        nc.scalar.activation(out=vrow[:], in_=vrow[:], func=AF.Sqrt,
                                             bias=eps_t[:])
                        nc.vector.reciprocal(out=vrow[:], in_=vrow[:])
                        mrow16 = lnsm.tile([1, S], F16, tag="mrow16", name=f"mrow16{idx}")
                        nc.scalar.activation(out=mrow16[:], in_=mrow[:], func=AF.Copy)
                        vrow16 = lnsm.tile([1, S], F16, tag="vrow16", name=f"vrow16{idx}")
                        nc.scalar.activation(out=vrow16[:], in_=vrow[:], func=AF.Copy)
                        meanB = stage.tile([128, S], F16, tag="lnB", name=f"meanB{idx}")
                        nc.gpsimd.partition_broadcast(meanB[:], mrow16[:])
                        rstdB = stage.tile([128, S], F16, tag="lnB", name=f"rstdB{idx}")
                        nc.gpsimd.partition_broadcast(rstdB[:], vrow16[:])
                        for kc in range(KC):
                            dte = stage.tile([128, S], F16, tag="lnd", name=f"lnd{idx}_{kc}")
                            eng = nc.vector if kc % 2 else nc.gpsimd
                            eng.tensor_sub(out=dte[:], in0=resT_t[:, kc, :],
                                           in1=meanB[:])
                            nc.vector.scalar_tensor_tensor(
                                out=dte[:], in0=dte[:], scalar=g_ap[:, kc:kc + 1],
                                in1=rstdB[:], op0=OP.mult, op1=OP.mult)
                            if outT is not None:
                                nc.scalar.activation(out=outT[:, kc, :], in_=dte[:],
                                                     func=AF.Identity,
                                                     bias=b_ap[:, kc:kc + 1])
                                if outT8 is not None:
                                    nc.scalar.activation(out=outT8[:, kc, :],
                                                         in_=dte[:],
                                                         func=AF.Identity,
                                                         bias=b_ap[:, kc:kc + 1])
                            else:
                                ote = stage.tile([128, S], F16, tag="lno",
                                                 name=f"lno{idx}_{kc}")
                                nc.scalar.activation(out=ote[:], in_=dte[:],
                                                     func=AF.Identity,
                                                     bias=b_ap[:, kc:kc + 1])
                                nc.scalar.dma_start(out=outT_d[:, kc, :], in_=ote[:])

                def emit_ffn():
                    """resT_t = relu(o2T @ w1 + b1) @ w2 + b2 + o2T  (fp8 DoubleRow)"""
                    with tc.tile_pool(name="ffn", bufs=1) as fp, \
                         tc.tile_pool(name="ffn_w", bufs=3) as fw, \
                         tc.tile_pool(name="ffn_w2", bufs=2) as fw2, \
                         tc.tile_pool(name="ffn_st", bufs=3) as fst, \
                         tc.tile_pool(name="ffn_ps", bufs=4, space="PSUM") as pf:
                        hT = fp.tile([128, FC, S], F16, tag="hT", name="hT")
                        for g in range(FC // 4):
                            wt = fw.tile([128, 4, KC, 128], F16, tag="w1t", name=f"w1_{g}")
                            nc.sync.dma_start(out=wt[:], in_=w1_d[4 * g:4 * (g + 1), :, :, :]
                                              .rearrange("mc p kc m -> p mc kc m"))
                            for mi in range(4):
                                mc = 4 * g + mi
                                ph = pf.tile([128, S], F32, tag="pf", name=f"ph{mc}")
                                for kc in range(KC):
                                    nc.tensor.matmul(ph[:], wt[:, mi, kc, :],
                                                     o2T_t[:, kc, :],
                                                     start=(kc == 0), stop=(kc == KC - 1))
                                nc.scalar.activation(out=hT[:, mc, :], in_=ph[:],
                                                     func=AF.Relu, bias=b1_t[:, mc:mc + 1])
                        for mc in range(KC):
                            wt = fw2.tile([128, FC, 128], F16, tag="w2t", name=f"w2_{mc}")
                            nc.sync.dma_start(out=wt[:], in_=w2_d[mc, :, :, :])
                            po = pf.tile([128, S], F32, tag="pf", name=f"po2_{mc}")
                            for kc in range(FC):
                                nc.tensor.matmul(po[:], wt[:, kc, :], hT[:, kc, :],
                                                 start=(kc == 0), stop=(kc == FC - 1))
                            # resT = (po + b2) + o2T
                            nc.vector.scalar_tensor_tensor(
                                out=resT_t[:, mc, :], in0=po[:], scalar=b2_t[:, mc:mc + 1],
                                in1=o2T_t[:, mc, :], op0=OP.add, op1=OP.add)

                # ---------------- main flow ----------------
                from collections import deque
                with tc.tile_pool(name="maskp", bufs=1) as maskp, \
                     tc.tile_pool(name="attn_t", bufs=1) as atp, \
                     tc.tile_pool(name="attn_w", bufs=3) as wpool:
                    maskT_t = maskp.tile([128, SC, S], F16, tag="maskT", name="maskT_t")
                    nc.scalar.dma_start(out=maskT_t[:], in_=maskT_d[:, :, :])
                    nc.scalar.dma_start(out=xzT8_t[:], in_=xzT8_d[:, :, :])
                    KT0 = atp.tile([128, KC, S], F16, tag="KT0", name="KT0")
                    Vt0 = atp.tile([128, SC, H, HD + 1], F8, tag="Vt0", name="Vt0")
                    KT1 = atp.tile([128, KC, S], F16, tag="KT1", name="KT1")
                    Vt1 = atp.tile([128, SC, H, HD + 1], F8, tag="Vt1", name="Vt1")

                    kv0 = make_kv_thunks(0, yzT8_t, wdec["wk_s"], wdec["wv_s"],
                                         KT0, Vt0, wpool)
                    kv1 = deque(make_kv_thunks(1, xzT8_t, wdec["wk_c"],
                                               wdec["wv_c"], KT1, Vt1, wpool))
                    if phases >= 1:
                        emit_attention(0, yzT8_t, wdec["wq_s"], wdec["wo_s"],
                                       True, yzT_t, KT0, Vt0, kv0, kv1, wpool)
                    if phases >= 2:
                        emit_ln(0, ln_t["g0"], ln_t["be0"], o1T_t, o1T8_t)
                    if phases >= 3:
                        emit_attention(1, o1T8_t, wdec["wq_c"], wdec["wo_c"],
                                       False, o1T_t, KT1, Vt1, list(kv1), deque(),
                                       wpool)
                    if phases >= 4:
                        emit_ln(1, ln_t["g1"], ln_t["be1"], o2T_t)
                if phases >= 5:
                    emit_ffn()
                if phases >= 6:
                    emit_ln(2, ln_t["g2"], ln_t["be2"], None)


            if loop_n == 1:
                _iter_body()
            else:
                with tc.For_i(0, loop_n, 1) as _it:
                    _iter_body(_it)

    nc.compile()
    return nc


def _get_nc():
    if "nc" not in _CACHE:
        _CACHE["nc"] = _build_nc()
    return _CACHE["nc"]


def _stat_blocks(W, mc_n, kc_n, dtype=np.float16):
    """[K, M] weight -> [MC, 128, KC, 128] blocked stationary layout:
    block[mc][p][kc][m] = W[kc*128+p, mc*128+m]."""
    W4 = np.ascontiguousarray(W, dtype=dtype).reshape(kc_n, 128, mc_n, 128)
    return np.ascontiguousarray(W4.transpose(2, 1, 0, 3))


def _featmaj(x, dtype=np.float16):
    """[T, D] -> [128, D/128, T] feature-major tile layout."""
    xT = np.ascontiguousarray(np.asarray(x, np.float32).astype(dtype)).T  # [D, T]
    d, t = xT.shape
    return np.ascontiguousarray(xT.reshape(d // 128, 128, t).transpose(1, 0, 2))


def prepare_inputs(xz, yz, wq_s, wk_s, wv_s, wo_s, wq_c, wk_c, wv_c, wo_c,
                   ffn_w1, ffn_b1, ffn_w2, ffn_b2,
                   ln0_g, ln0_b, ln1_g, ln1_b, ln2_g, ln2_b,
                   look_ahead_mask, pad_mask):
    import ml_dtypes
    F8NP = ml_dtypes.float8_e4m3

    def headcat(w):  # [H, D, HD] -> [D, H*HD]
        return np.ascontiguousarray(
            np.transpose(np.asarray(w, np.float32), (1, 0, 2)).reshape(D, D))

    shared = {
        "wq_s": _stat_blocks(headcat(wq_s) * WSCALE, KC, KC, F8NP),
        "wk_s": _stat_blocks(headcat(wk_s) * WSCALE, KC, KC, F8NP),
        "wo_s": _stat_blocks(np.asarray(wo_s, np.float32), KC, KC),
        "wq_c": _stat_blocks(headcat(wq_c) * WSCALE, KC, KC, F8NP),
        "wk_c": _stat_blocks(headcat(wk_c) * WSCALE, KC, KC, F8NP),
        "wo_c": _stat_blocks(np.asarray(wo_c, np.float32), KC, KC),
        "wv_s": np.ascontiguousarray(
            (headcat(wv_s) * WSCALE).astype(F8NP).reshape(KC, 128, D)),
        "wv_c": np.ascontiguousarray(
            (headcat(wv_c) * WSCALE).astype(F8NP).reshape(KC, 128, D)),
        "w1": _stat_blocks(np.asarray(ffn_w1, np.float32), FC, KC),
        "w2": _stat_blocks(np.asarray(ffn_w2, np.float32), KC, FC),
    }
    # additive mask, transposed to [s, t], tiled [128, SC, S]: 0 or NEGBIG
    m = np.asarray(look_ahead_mask, np.float32)[0, 0]  # [t, s]
    mT = (m.T * np.float32(NEGBIG)).astype(np.float16)  # [s, t], {0, NEGBIG}
    shared["maskT"] = np.ascontiguousarray(
        mT.reshape(SC, 128, S).transpose(1, 0, 2))

    def col8(v):
        return np.asarray(v, np.float32).reshape(KC, 128).T
    const_common = [np.asarray(ffn_b1, np.float32).reshape(FC, 128).T,
                    col8(ffn_b2), col8(ln0_g), col8(ln0_b), col8(ln1_g),
                    col8(ln1_b), col8(ln2_g), col8(ln2_b)]
    in_maps = []
    for c in range(B):
        im = dict(shared)
        im["yzT"] = _featmaj(np.asarray(yz, np.float32)[c])
        im["yzT8"] = _featmaj(np.asarray(yz, np.float32)[c], F8NP)
        im["xzT8"] = _featmaj(np.asarray(xz, np.float32)[c], F8NP)
        pb = (np.asarray(pad_mask, np.float32)[c, 0, 0] * NEGBIG).astype(np.float32)
        im["consts"] = np.ascontiguousarray(np.concatenate(
            const_common + [pb.reshape(SC, 128).T], axis=1, dtype=np.float32))
        in_maps.append(im)
    return in_maps


def kernel(**inputs):
    from concourse.bass_utils import run_bass_kernel_spmd

    nc = _get_nc()
    in_maps = prepare_inputs(**inputs)
    res = run_bass_kernel_spmd(nc, in_maps, core_ids=list(range(B)))
    out = np.empty((B, S, D), np.float32)
    for c in range(B):
        oT = res.results[c]["outT"].astype(np.float32)  # [128, KC, S]
        out[c] = oT.transpose(1, 0, 2).reshape(D, S).T
    return out

